# revision 45
# baseline (speedup 1.0000x reference)
"""Trainium2 Bass kernel for FovConv2dCont (per-pixel foveated Gaussian blur + 5x5 conv).

kernel(**inputs): takes FULL inputs
  input_data f32 (8,3,224,224), foa_xy int (8,2), weight f32 (64,3,5,5)
returns f32 (8,64,224,224). Batch is data-parallel across 8 NeuronCores (1 sample/core).

Math (exact identities; bf16 storage on the heavy elementwise chain):
  gaussian tap exp(-(i^2+j^2)/(2 s^2)) = u^(i^2) * u^(j^2),  u = exp(-1/(2 s^2))
  normalizer sum over 7x7 taps = (1 + 2u + 2u^4 + 2u^9)^2
  numerator = sum_e u^e S_e with S_e the sum of taps at squared radius e
  (terms e=13,18 dropped: bounded by ~2e-3 relative, below bf16 noise)
  m = numerator / norm ; y = conv5x5(m, w) with K=54 im2col whose partitions
  are (ci, dy' in 0..5, dxg in {0,2,4}) and two accumulating matmuls
  (wtA: even dx, wtB: odd dx via +1 column offset); lhsT columns = 2*OC so
  each matmul column yields TWO output rows (even rows on psum partitions
  0-63, odd on 64-127).

Pipeline structure:
  - sigma/u^e coefficient chain hoisted before the modulation; it only needs
    the row/col distance vectors, so it overlaps the xs input load
  - gaussian modulation is one full-width pass (DVE is partition-parallel, so
    partition-splitting it doubles vector time); ~6 independent ops ride
    gpsimd to shorten the vector critical path
  - m bounces through a CHANNEL-MAJOR DRAM buffer mf[C,230,MW] (3 per-channel
    writes with 912B runs + 1 pad write), so each icf im2col partition is ONE
    contiguous DRAM run: 9 one-hop DMAs per strip (per ci,dxg), issues split
    sync/gpsimd. NOTE: SBUF DMA APs may hop partitions ONLY in dim 0 --
    non-first partition-hop dims silently corrupt on HW.
  - psum->stage copies alternate scalar(4)/vector(3); outs on scalar
"""

import os
import sys

sys.path.insert(0, "/opt/trn_rl_repo")

import numpy as np
import ml_dtypes

def _ensure_ntff_hook():
    """Register the NTFF profile hook if the image's antenv lacks axon_hooks
    (needed only for trace=True timing runs; harmless otherwise)."""
    try:
        import antenv.axon_hooks  # noqa: F401
        return
    except ImportError:
        pass
    try:
        import types
        import antenv
        import importlib.util as ilu

        spec = ilu.spec_from_file_location(
            "trn_agent_boot.trn_boot", "/root/.axon_site/trn_agent_boot/trn_boot.py"
        )
        mod = types.ModuleType("antenv.axon_hooks")
        _hook_holder = {"hook": None}

        def set_axon_ntff_profile_hook(h):
            _hook_holder["hook"] = h

        def get_axon_ntff_profile_hook():
            return _hook_holder["hook"]

        mod.set_axon_ntff_profile_hook = set_axon_ntff_profile_hook
        mod.get_axon_ntff_profile_hook = get_axon_ntff_profile_hook
        sys.modules["antenv.axon_hooks"] = mod
        antenv.axon_hooks = mod

        boot = ilu.module_from_spec(spec)
        spec.loader.exec_module(boot)
        hook = boot._ntff_profile_via_ctypes("/opt/axon/libaxon_pjrt.so")
        set_axon_ntff_profile_hook(hook)
    except Exception:
        pass


_ensure_ntff_hook()

import concourse.bass as bass
import concourse.bacc as bacc_mod
import concourse.mybir as mybir
from concourse.bass_utils import run_bass_kernel_spmd
from concourse.tile import TileContext
from concourse.alu_op_type import AluOpType

F32 = mybir.dt.float32
BF16 = mybir.dt.bfloat16
AF = mybir.ActivationFunctionType

H = W = 224
C = 3
OC = 64
KG = 7
PG = KG // 2            # 3
KC = 5
PC = KC // 2            # 2
WP = W + 2 * PG         # 230
SR = 8                  # strip rows per partition of xs
NP = H // 2             # 112 partitions
MW = W + 2 * PC         # 228
DNORM = float(np.sqrt(H * H + W * W))

NDY = KC + 1            # 6 dy' values (row-pair trick)
NDXG = 3                # even-dx groups (dx = 0,2,4)
K54 = C * NDY * NDXG    # 54 contraction size
RS = 56                 # output rows per conv strip
NSTRIP = H // RS        # 4
FSIF = RS * MW          # 12768 icf free size
RUNF = (RS - 2) * MW + W  # 12536 elems read per (ci,dy',dxg)
FSST = RS * W // 4      # 3136 stage free size (per parity, per half-strip)
NCH = RS // 4           # 14 matmul chunks per strip (4 rows each)
NMM = 2 * W             # 448 matmul N (2 even/odd row pairs x 224)
MROWS = 230             # mf bounce rows (padded)

EXPS_PAIR = {1: (0, 1), 4: (0, 2), 5: (1, 2), 9: (0, 3)}
EXPS_DIAG = {2: 1, 8: 2}
ALL_E = sorted(set(EXPS_PAIR) | set(EXPS_DIAG))

LAST_RESULTS = None
_CACHED = None


def _v(ap_src, offset_elems, dims):
    """Raw strided (possibly overlapping/broadcast) view of a flat AP.
    dims = [(step, count), ...]; for SBUF/PSUM the first dim (and only the
    first) may hop partitions (step = k * free_size)."""
    fv = ap_src.flatten()
    v = fv.copy()
    v.offset = fv.offset + offset_elems
    v.ap = mybir.VecI64Pair([list(d) for d in dims])
    return v


def _build_nc():
    nc = bacc_mod.Bacc()

    xp = nc.declare_dram_parameter("xp", [C, WP, WP], BF16, isOutput=False)
    av = nc.declare_dram_parameter("av", [H], F32, isOutput=False)
    bv = nc.declare_dram_parameter("bv", [H], F32, isOutput=False)
    wc = nc.declare_dram_parameter("wc", [2 * K54, 2 * OC], BF16, isOutput=False)
    # parity-major output: [parity, oc, row_pair, col]. Each (parity, oc)
    # strip-block is one contiguous 3136-elem run (6.3KB packets instead of
    # 448B); the host interleaves parities back into [OC, H, W].
    out = nc.declare_dram_parameter("out", [2, OC, H // 2, W], BF16,
                                    isOutput=True)

    with TileContext(nc) as tc:
        with (
            tc.tile_pool(name="pers", bufs=1) as pers,
            tc.tile_pool(name="psum", bufs=8, space="PSUM") as psum_pool,
            tc.tile_pool(name="icf", bufs=3) as icf_pool,
            tc.tile_pool(name="stg", bufs=3) as stg_pool,
            tc.tile_pool(name="dram", bufs=1, space="DRAM") as dram_pool,
        ):
            XFS = C * SR * WP                       # xs free size 5520
            xs = pers.tile([NP, XFS], BF16)
            CFS = 2 * W                             # coeff free size 448
            at = pers.tile([NP, 2], F32)
            bvf = pers.tile([NP, W], F32)
            d2 = pers.tile([NP, CFS], F32)
            dist = pers.tile([NP, CFS], F32)
            sig = pers.tile([NP, CFS], F32)
            sqv = pers.tile([NP, CFS], F32)
            isg = pers.tile([NP, CFS], F32)
            u1f = pers.tile([NP, CFS], F32)
            u4f = pers.tile([NP, CFS], F32)
            u9f = pers.tile([NP, CFS], F32)
            t1 = pers.tile([NP, CFS], F32)
            t2 = pers.tile([NP, CFS], F32)
            sfield = pers.tile([NP, CFS], F32)
            rsf = pers.tile([NP, CFS], F32)
            rb = pers.tile([NP, CFS], BF16)
            ub = {e: pers.tile([NP, CFS], BF16, name=f"ub{e}") for e in ALL_E}
            RFS = C * 2 * WP                        # rowpair free size 1380
            rp = {a: pers.tile([NP, RFS], BF16, name=f"rp{a}") for a in (1, 2, 3)}
            PFS = C * 2 * W                         # P tile free size 1344
            ptiles = {}
            for e, (a, b) in EXPS_PAIR.items():
                ptiles[(a, b)] = pers.tile([NP, PFS], BF16, name=f"p{a}{b}")
                if a != 0:
                    ptiles[(b, a)] = pers.tile([NP, PFS], BF16, name=f"p{b}{a}")
            for e, a in EXPS_DIAG.items():
                ptiles[(a, a)] = pers.tile([NP, PFS], BF16, name=f"pd{a}")
            qtiles = {e: pers.tile([NP, PFS], BF16, name=f"q{e}") for e in EXPS_PAIR}
            prod = pers.tile([NP, PFS], BF16)
            acc = pers.tile([NP, PFS], BF16)
            acc2 = pers.tile([NP, PFS], BF16)
            npr5 = pers.tile([NP, PFS], BF16)
            npr8 = pers.tile([NP, PFS], BF16)
            npr9 = pers.tile([NP, PFS], BF16)
            MFS = C * 2 * MW                        # m free size 1368
            # mt2 free layout (ci, rh, col): flat = ci*456 + rh*228 + col;
            # partition p holds m rows 2p, 2p+1 (= padded rows 2p+2, 2p+3);
            # partitions 112,113 stay zero (bottom pad rows 226..229); cols
            # 0,1,226,227 stay zero (left/right pad)
            mt2 = pers.tile([NP + 2, MFS], BF16)
            wtA = pers.tile([K54, 2 * OC], BF16)
            wtB = pers.tile([K54, 2 * OC], BF16)
            # channel-major padded-m bounce buffer in DRAM: rows contiguous
            # per channel so each icf partition is ONE long run
            mf = dram_pool.tile([C, MROWS, MW], BF16)

            # ---------------- loads + zero fills ----------------
            nc.scalar.dma_start(
                out=_v(at[:], 0, [[2, NP], [1, 2]]),
                in_=_v(av[:], 0, [[2, NP], [1, 2]]),
            )
            nc.scalar.dma_start(
                out=_v(bvf[:], 0, [[W, NP], [1, W]]),
                in_=_v(bv[:], 0, [[0, NP], [1, W]]),
            )
            for ci in range(C):
                src = _v(xp[ci], 0, [[2 * WP, NP], [1, SR * WP]])
                dst = _v(xs[:], ci * SR * WP, [[XFS, NP], [1, SR * WP]])
                nc.sync.dma_start(out=dst, in_=src)

            nc.gpsimd.memset(mt2[:], 0.0)
            nc.scalar.dma_start(
                out=_v(wtA[:], 0, [[2 * OC, K54], [1, 2 * OC]]),
                in_=_v(wc[:], 0, [[2 * OC, K54], [1, 2 * OC]]),
            )
            nc.scalar.dma_start(
                out=_v(wtB[:], 0, [[2 * OC, K54], [1, 2 * OC]]),
                in_=_v(wc[:], K54 * 2 * OC, [[2 * OC, K54], [1, 2 * OC]]),
            )
            # mf pad rows 0,1 per channel: zeros from never-written mt2
            # partition 112 (3 x 456-elem chunks of its zeroed free space)
            nc.scalar.dma_start(
                out=_v(mf[:], 0, [[MROWS * MW, C], [1, 2 * MW]]),
                in_=_v(mt2[:], 112 * MFS, [[MFS, 1], [2 * MW, C], [1, 2 * MW]]),
            )

            # ---------------- coefficient chain (no xs dependency) ----------
            hs = slice(0, NP)
            for rh in range(2):
                nc.vector.tensor_scalar(
                    d2[hs, rh * W:(rh + 1) * W], bvf[hs, :],
                    at[hs, rh:rh + 1], None, AluOpType.add,
                )
            nc.scalar.activation(dist[hs, :], d2[hs, :], AF.Sqrt)
            nc.scalar.activation(sig[hs, :], dist[hs, :], AF.Copy,
                                 bias=0.01, scale=0.99)
            nc.scalar.activation(sqv[hs, :], sig[hs, :], AF.Square)
            nc.vector.reciprocal_approx_fast(isg[:], sqv[:])
            nc.scalar.activation(u1f[hs, :], isg[hs, :], AF.Exp, scale=-0.5)
            nc.scalar.activation(u4f[hs, :], isg[hs, :], AF.Exp, scale=-2.0)
            nc.scalar.activation(u9f[hs, :], isg[hs, :], AF.Exp, scale=-4.5)
            for e in ALL_E:
                nc.scalar.activation(ub[e][hs, :], isg[hs, :], AF.Exp,
                                     scale=-0.5 * e)
            nc.vector.tensor_tensor(t1[hs, :], u1f[hs, :], u4f[hs, :],
                                    AluOpType.add)
            nc.vector.tensor_tensor(t2[hs, :], t1[hs, :], u9f[hs, :],
                                    AluOpType.add)
            nc.vector.tensor_scalar(
                sfield[hs, :], t2[hs, :], 2.0, 1.0,
                AluOpType.mult, AluOpType.add
            )
            nc.vector.reciprocal_approx_fast(rsf[:], sfield[:])
            nc.scalar.activation(rb[hs, :], rsf[hs, :], AF.Square)

            # ---------------- gaussian modulation (full width) --------------
            def gauss():
                P0, PN = 0, NP

                def xsv(col_off):
                    return _v(xs[:], P0 * XFS + PG * WP + PG + col_off,
                              [[XFS, PN], [SR * WP, C], [WP, 2], [1, W]])

                def rpv(a, col_off):
                    return _v(rp[a][:], P0 * RFS + PG + col_off,
                              [[RFS, PN], [2 * WP, C], [WP, 2], [1, W]])

                def pv(t):
                    return _v(t[:], P0 * PFS, [[PFS, PN], [1, PFS]])

                def pv4(t):
                    return _v(t[:], P0 * PFS,
                              [[PFS, PN], [2 * W, C], [W, 2], [1, W]])

                def uv(t):
                    return _v(t[:], P0 * CFS, [[CFS, PN], [0, C], [1, CFS]])

                hs = slice(P0, P0 + PN)

                for a in (1, 2, 3):
                    i0 = _v(xs[:], P0 * XFS + (PG - a) * WP,
                            [[XFS, PN], [SR * WP, C], [1, 2 * WP]])
                    i1 = _v(xs[:], P0 * XFS + (PG + a) * WP,
                            [[XFS, PN], [SR * WP, C], [1, 2 * WP]])
                    o = _v(rp[a][:], P0 * RFS, [[RFS, PN], [1, RFS]])
                    nc.vector.tensor_tensor(o, i0, i1, AluOpType.add)

                for (a, b), pt in ptiles.items():
                    if b == 0:
                        continue
                    if a == 0:
                        i0, i1 = xsv(-b), xsv(+b)
                    else:
                        i0, i1 = rpv(a, -b), rpv(a, +b)
                    # off-critical-chain P adds ride gpsimd (3.4x slower but
                    # it idles while vector grinds the rest)
                    eng = (nc.gpsimd
                           if (a, b) in ((0, 2), (0, 3), (1, 2), (2, 2))
                           else nc.vector)
                    eng.tensor_tensor(pv4(pt), i0, i1, AluOpType.add)

                for e, (a, b) in EXPS_PAIR.items():
                    if a == 0:
                        nc.vector.tensor_tensor(
                            pv4(qtiles[e]), pv4(ptiles[(a, b)]), rpv(b, 0),
                            AluOpType.add,
                        )
                    else:
                        nc.vector.tensor_tensor(
                            pv(qtiles[e]), pv(ptiles[(a, b)]),
                            pv(ptiles[(b, a)]), AluOpType.add,
                        )

                sto = {1: qtiles[1], 2: ptiles[(1, 1)], 4: qtiles[4],
                       5: qtiles[5], 8: ptiles[(2, 2)], 9: qtiles[9]}
                prt = {1: prod, 2: acc, 4: acc2, 5: npr5, 8: npr8, 9: npr9}
                for e in ALL_E:
                    nc.vector.tensor_tensor(pv(prt[e]), uv(ub[e]), pv(sto[e]),
                                            AluOpType.mult)
                nc.vector.tensor_tensor(pv(qtiles[1]), pv(prt[1]), pv(prt[2]),
                                        AluOpType.add)
                nc.vector.tensor_tensor(pv(qtiles[4]), pv(prt[4]), pv(prt[5]),
                                        AluOpType.add)
                nc.vector.tensor_tensor(pv4(qtiles[5]), xsv(0), pv4(prt[8]),
                                        AluOpType.add)
                nc.vector.tensor_tensor(pv(qtiles[9]), pv(qtiles[1]),
                                        pv(qtiles[4]), AluOpType.add)
                nc.vector.tensor_tensor(pv(ptiles[(1, 1)]), pv(qtiles[5]),
                                        pv(prt[9]), AluOpType.add)
                nc.vector.tensor_tensor(pv(prod), pv(qtiles[9]),
                                        pv(ptiles[(1, 1)]), AluOpType.add)

                # m row pairs into mt2, (ci, rh, col) free order
                mdst = _v(mt2[:], P0 * MFS + PC,
                          [[MFS, PN], [2 * MW, C], [MW, 2], [1, W]])
                nc.vector.tensor_tensor(mdst, uv(rb), pv(prod), AluOpType.mult)

            def conv_strip(g):
                # one-hop im2col: icf[p=(dyp*C+ci)*NDXG+dxg] <- one contiguous
                # RUNF+1 run of channel-major mf. 9 DMAs (per ci,dxg): SBUF
                # dst hops partitions only in dim 0 (stride 9 partitions over
                # dyp); issues split across sync and gpsimd queues.
                icf = icf_pool.tile([K54, FSIF], BF16, name="icf")
                for ci in range(C):
                    for dxg in range(NDXG):
                        q = nc.sync if (ci * NDXG + dxg) % 2 == 0 else nc.gpsimd
                        q.dma_start(
                            out=_v(icf[:], (ci * NDXG + dxg) * FSIF,
                                   [[C * NDXG * FSIF, NDY], [1, RUNF + 1]]),
                            in_=_v(mf[:],
                                   (ci * MROWS + g * RS) * MW + 2 * dxg,
                                   [[MW, NDY], [1, RUNF + 1]]),
                        )

                for h2 in range(2):
                    stage = stg_pool.tile([2 * OC, FSST], BF16, name="stage")
                    for ch in range(NCH // 2):
                        c = h2 * (NCH // 2) + ch
                        ps = psum_pool.tile([2 * OC, NMM], F32, name="ps")
                        rhs0 = _v(icf[:], 4 * c * MW,
                                  [[FSIF, K54], [2 * MW, 2], [1, W]])
                        rhs1 = _v(icf[:], 4 * c * MW + 1,
                                  [[FSIF, K54], [2 * MW, 2], [1, W]])
                        nc.tensor.matmul(ps[:], wtA[:], rhs0,
                                         start=True, stop=False)
                        nc.tensor.matmul(ps[:], wtB[:], rhs1,
                                         start=False, stop=True)
                        st_slice = stage[:, ch * NMM:(ch + 1) * NMM]
                        if ch % 2 == 0:
                            nc.scalar.copy(st_slice, ps[:])
                        else:
                            nc.vector.tensor_scalar(
                                st_slice, ps[:], 1.0, None, AluOpType.mult
                            )

                    rp0 = g * (RS // 4) * 2 + h2 * (RS // 4)
                    for h in range(2):
                        dst = _v(out[:],
                                 h * OC * (H // 2) * W + rp0 * W,
                                 [[(H // 2) * W, OC], [1, FSST]])
                        src = _v(stage[:], h * OC * FSST,
                                 [[FSST, OC], [1, FSST]])
                        nc.scalar.dma_start(out=dst, in_=src)

            gauss()

            # mt2 -> channel-major mf rows 2..229 (incl. zero bottom pad from
            # partitions 112,113); per-ci so each partition's (rh,col) block
            # is one contiguous 456-elem run
            for ci in range(C):
                nc.sync.dma_start(
                    out=_v(mf[:], (ci * MROWS + 2) * MW,
                           [[2 * MW, NP + 2], [1, 2 * MW]]),
                    in_=_v(mt2[:], ci * 2 * MW,
                           [[MFS, NP + 2], [1, 2 * MW]]),
                )

            for g in range(NSTRIP):
                conv_strip(g)

    return nc


def _get_nc():
    global _CACHED
    if _CACHED is None:
        nc = _build_nc()
        nc.finalize()
        _CACHED = nc
    return _CACHED


def _host_prep(input_data, foa_xy, weight):
    b = input_data.shape[0]
    wcs = np.zeros((2 * K54, 2 * OC), dtype=np.float32)
    for ci in range(C):
        for dyp in range(NDY):
            for dxg in range(NDXG):
                k = (dyp * C + ci) * NDXG + dxg
                if dyp <= 4:
                    wcs[k, :OC] = weight[:, ci, dyp, 2 * dxg]
                    if dxg <= 1:
                        wcs[K54 + k, :OC] = weight[:, ci, dyp, 2 * dxg + 1]
                if dyp >= 1:
                    wcs[k, OC:] = weight[:, ci, dyp - 1, 2 * dxg]
                    if dxg <= 1:
                        wcs[K54 + k, OC:] = weight[:, ci, dyp - 1, 2 * dxg + 1]
    wcs = wcs.astype(ml_dtypes.bfloat16)
    idx = np.arange(H, dtype=np.float64)
    in_maps = []
    for i in range(b):
        xpad = np.zeros((C, WP, WP), dtype=ml_dtypes.bfloat16)
        xpad[:, PG:PG + H, PG:PG + W] = input_data[i].astype(ml_dtypes.bfloat16)
        fx, fy = float(foa_xy[i, 0]), float(foa_xy[i, 1])
        a_sq = (((idx - fx) / DNORM) ** 2).astype(np.float32)
        b_sq = (((idx - fy) / DNORM) ** 2).astype(np.float32)
        in_maps.append({"xp": xpad, "av": a_sq, "bv": b_sq, "wc": wcs})
    return in_maps


def kernel(input_data, foa_xy, weight):
    global LAST_RESULTS
    nc = _get_nc()
    in_maps = _host_prep(np.asarray(input_data), np.asarray(foa_xy),
                         np.asarray(weight))
    trace = bool(int(os.environ.get("BASSKERNEL_TRACE", "0")))
    res = run_bass_kernel_spmd(nc, in_maps, core_ids=list(range(8)), trace=trace)
    LAST_RESULTS = res
    outs = []
    for r in res.results:
        o2 = np.asarray(r["out"], dtype=np.float32)  # [2, OC, H/2, W]
        full = np.empty((OC, H, W), dtype=np.float32)
        full[:, 0::2, :] = o2[0]
        full[:, 1::2, :] = o2[1]
        outs.append(full)
    return np.stack(outs, axis=0)


# revision 48
# speedup vs baseline: 1.1637x; 1.1637x over previous
"""Trainium2 Bass kernel for FovConv2dCont (per-pixel foveated Gaussian blur + 5x5 conv).

kernel(**inputs): takes FULL inputs
  input_data f32 (8,3,224,224), foa_xy int (8,2), weight f32 (64,3,5,5)
returns f32 (8,64,224,224). Batch is data-parallel across 8 NeuronCores (1 sample/core).

Math (exact identities; bf16 storage on the heavy elementwise chain):
  gaussian tap exp(-(i^2+j^2)/(2 s^2)) = u^(i^2) * u^(j^2),  u = exp(-1/(2 s^2))
  normalizer sum over 7x7 taps = (1 + 2u + 2u^4 + 2u^9)^2
  numerator = sum_e u^e S_e with S_e the sum of taps at squared radius e
  (terms e=13,18 dropped: bounded by ~2e-3 relative, below bf16 noise)
  m = numerator / norm ; y = conv5x5(m, w) with K=54 im2col whose partitions
  are (ci, dy' in 0..5, dxg in {0,2,4}) and two accumulating matmuls
  (wtA: even dx, wtB: odd dx via +1 column offset); lhsT columns = 2*OC so
  each matmul column yields TWO output rows (even rows on psum partitions
  0-63, odd on 64-127).

Pipeline structure:
  - sigma/u^e coefficient chain hoisted before the modulation; it only needs
    the row/col distance vectors, so it overlaps the xs input load
  - gaussian modulation is one full-width pass (DVE is partition-parallel, so
    partition-splitting it doubles vector time); ~6 independent ops ride
    gpsimd to shorten the vector critical path
  - m bounces through a CHANNEL-MAJOR DRAM buffer mf[C,230,MW] (3 per-channel
    writes with 912B runs + 1 pad write), so each icf im2col partition is ONE
    contiguous DRAM run: 9 one-hop DMAs per strip (per ci,dxg), issues split
    sync/gpsimd. NOTE: SBUF DMA APs may hop partitions ONLY in dim 0 --
    non-first partition-hop dims silently corrupt on HW.
  - psum->stage copies alternate scalar(4)/vector(3); outs on scalar
"""

import os
import sys

sys.path.insert(0, "/opt/trn_rl_repo")

import numpy as np
import ml_dtypes

def _ensure_ntff_hook():
    """Register the NTFF profile hook if the image's antenv lacks axon_hooks
    (needed only for trace=True timing runs; harmless otherwise)."""
    try:
        import antenv.axon_hooks  # noqa: F401
        return
    except ImportError:
        pass
    try:
        import types
        import antenv
        import importlib.util as ilu

        spec = ilu.spec_from_file_location(
            "trn_agent_boot.trn_boot", "/root/.axon_site/trn_agent_boot/trn_boot.py"
        )
        mod = types.ModuleType("antenv.axon_hooks")
        _hook_holder = {"hook": None}

        def set_axon_ntff_profile_hook(h):
            _hook_holder["hook"] = h

        def get_axon_ntff_profile_hook():
            return _hook_holder["hook"]

        mod.set_axon_ntff_profile_hook = set_axon_ntff_profile_hook
        mod.get_axon_ntff_profile_hook = get_axon_ntff_profile_hook
        sys.modules["antenv.axon_hooks"] = mod
        antenv.axon_hooks = mod

        boot = ilu.module_from_spec(spec)
        spec.loader.exec_module(boot)
        hook = boot._ntff_profile_via_ctypes("/opt/axon/libaxon_pjrt.so")
        set_axon_ntff_profile_hook(hook)
    except Exception:
        pass


_ensure_ntff_hook()

import concourse.bass as bass
import concourse.bacc as bacc_mod
import concourse.mybir as mybir
from concourse.bass_utils import run_bass_kernel_spmd
from concourse.tile import TileContext
from concourse.alu_op_type import AluOpType

F32 = mybir.dt.float32
BF16 = mybir.dt.bfloat16
AF = mybir.ActivationFunctionType

H = W = 224
C = 3
OC = 64
KG = 7
PG = KG // 2            # 3
KC = 5
PC = KC // 2            # 2
WP = W + 2 * PG         # 230
SR = 8                  # strip rows per partition of xs
NP = H // 2             # 112 partitions
MW = W + 2 * PC         # 228
DNORM = float(np.sqrt(H * H + W * W))

NDY = KC + 1            # 6 dy' values (row-pair trick)
NDXG = 3                # even-dx groups (dx = 0,2,4)
K54 = C * NDY * NDXG    # 54 contraction size
RS = 56                 # output rows per conv strip
NSTRIP = H // RS        # 4
FSIF = RS * MW          # 12768 icf free size
RUNF = (RS - 2) * MW + W  # 12536 elems read per (ci,dy',dxg)
FSST = RS * W // 4      # 3136 stage free size (per parity, per half-strip)
NCH = RS // 4           # 14 matmul chunks per strip (4 rows each)
NMM = 2 * W             # 448 matmul N (2 even/odd row pairs x 224)
MROWS = 230             # mf bounce rows (padded)

EXPS_PAIR = {1: (0, 1), 4: (0, 2), 5: (1, 2), 9: (0, 3)}
EXPS_DIAG = {2: 1, 8: 2}
ALL_E = sorted(set(EXPS_PAIR) | set(EXPS_DIAG))

LAST_RESULTS = None
_CACHED = None


def _v(ap_src, offset_elems, dims):
    """Raw strided (possibly overlapping/broadcast) view of a flat AP.
    dims = [(step, count), ...]; for SBUF/PSUM the first dim (and only the
    first) may hop partitions (step = k * free_size)."""
    fv = ap_src.flatten()
    v = fv.copy()
    v.offset = fv.offset + offset_elems
    v.ap = mybir.VecI64Pair([list(d) for d in dims])
    return v


def _build_nc():
    nc = bacc_mod.Bacc()

    xp = nc.declare_dram_parameter("xp", [C, WP, WP], BF16, isOutput=False)
    av = nc.declare_dram_parameter("av", [H], F32, isOutput=False)
    bv = nc.declare_dram_parameter("bv", [H], F32, isOutput=False)
    wc = nc.declare_dram_parameter("wc", [2 * K54, 2 * OC], BF16, isOutput=False)
    # parity-major output: [parity, oc, row_pair, col]. Each (parity, oc)
    # strip-block is one contiguous 3136-elem run (6.3KB packets instead of
    # 448B); the host interleaves parities back into [OC, H, W].
    out = nc.declare_dram_parameter("out", [2, OC, H // 2, W], BF16,
                                    isOutput=True)

    with TileContext(nc) as tc:
        with (
            tc.tile_pool(name="pers", bufs=1) as pers,
            tc.tile_pool(name="psum", bufs=8, space="PSUM") as psum_pool,
            tc.tile_pool(name="icf", bufs=3) as icf_pool,
            tc.tile_pool(name="stg", bufs=3) as stg_pool,
            tc.tile_pool(name="dram", bufs=1, space="DRAM") as dram_pool,
        ):
            XFS = C * SR * WP                       # xs free size 5520
            xs = pers.tile([NP, XFS], BF16)
            CFS = 2 * W                             # coeff free size 448
            at = pers.tile([NP, 2], F32)
            bvf = pers.tile([NP, W], F32)
            d2 = pers.tile([NP, CFS], F32)
            dist = pers.tile([NP, CFS], F32)
            sig = pers.tile([NP, CFS], F32)
            sqv = pers.tile([NP, CFS], F32)
            isg = pers.tile([NP, CFS], F32)
            u1f = pers.tile([NP, CFS], F32)
            u4f = pers.tile([NP, CFS], F32)
            u9f = pers.tile([NP, CFS], F32)
            t1 = pers.tile([NP, CFS], F32)
            t2 = pers.tile([NP, CFS], F32)
            sfield = pers.tile([NP, CFS], F32)
            rsf = pers.tile([NP, CFS], F32)
            rb = pers.tile([NP, CFS], BF16)
            ub = {e: pers.tile([NP, CFS], BF16, name=f"ub{e}") for e in ALL_E}
            RFS = C * 2 * WP                        # rowpair free size 1380
            rp = {a: pers.tile([NP, RFS], BF16, name=f"rp{a}") for a in (1, 2, 3)}
            PFS = C * 2 * W                         # P tile free size 1344
            ptiles = {}
            for e, (a, b) in EXPS_PAIR.items():
                ptiles[(a, b)] = pers.tile([NP, PFS], BF16, name=f"p{a}{b}")
                if a != 0:
                    ptiles[(b, a)] = pers.tile([NP, PFS], BF16, name=f"p{b}{a}")
            for e, a in EXPS_DIAG.items():
                ptiles[(a, a)] = pers.tile([NP, PFS], BF16, name=f"pd{a}")
            qtiles = {e: pers.tile([NP, PFS], BF16, name=f"q{e}") for e in EXPS_PAIR}
            prod = pers.tile([NP, PFS], BF16)
            acc = pers.tile([NP, PFS], BF16)
            acc2 = pers.tile([NP, PFS], BF16)
            npr5 = pers.tile([NP, PFS], BF16)
            npr8 = pers.tile([NP, PFS], BF16)
            npr9 = pers.tile([NP, PFS], BF16)
            MFS = C * 2 * MW                        # m free size 1368
            # mt2 free layout (ci, rh, col): flat = ci*456 + rh*228 + col;
            # partition p holds m rows 2p, 2p+1 (= padded rows 2p+2, 2p+3);
            # partitions 112,113 stay zero (bottom pad rows 226..229); cols
            # 0,1,226,227 stay zero (left/right pad)
            mt2 = pers.tile([NP + 2, MFS], BF16)
            wtA = pers.tile([K54, 2 * OC], BF16)
            wtB = pers.tile([K54, 2 * OC], BF16)
            # channel-major padded-m bounce buffer in DRAM: rows contiguous
            # per channel so each icf partition is ONE long run
            mf = dram_pool.tile([C, MROWS, MW], BF16)

            # ---------------- loads + zero fills ----------------
            nc.scalar.dma_start(
                out=_v(at[:], 0, [[2, NP], [1, 2]]),
                in_=_v(av[:], 0, [[2, NP], [1, 2]]),
            )
            nc.scalar.dma_start(
                out=_v(bvf[:], 0, [[W, NP], [1, W]]),
                in_=_v(bv[:], 0, [[0, NP], [1, W]]),
            )
            for ci in range(C):
                src = _v(xp[ci], 0, [[2 * WP, NP], [1, SR * WP]])
                dst = _v(xs[:], ci * SR * WP, [[XFS, NP], [1, SR * WP]])
                nc.sync.dma_start(out=dst, in_=src)

            nc.gpsimd.memset(mt2[:], 0.0)
            nc.scalar.dma_start(
                out=_v(wtA[:], 0, [[2 * OC, K54], [1, 2 * OC]]),
                in_=_v(wc[:], 0, [[2 * OC, K54], [1, 2 * OC]]),
            )
            nc.scalar.dma_start(
                out=_v(wtB[:], 0, [[2 * OC, K54], [1, 2 * OC]]),
                in_=_v(wc[:], K54 * 2 * OC, [[2 * OC, K54], [1, 2 * OC]]),
            )
            # mf pad rows 0,1 per channel: zeros from never-written mt2
            # partition 112 (3 x 456-elem chunks of its zeroed free space)
            nc.scalar.dma_start(
                out=_v(mf[:], 0, [[MROWS * MW, C], [1, 2 * MW]]),
                in_=_v(mt2[:], 112 * MFS, [[MFS, 1], [2 * MW, C], [1, 2 * MW]]),
            )

            # ---------------- coefficient chain (no xs dependency) ----------
            hs = slice(0, NP)
            for rh in range(2):
                nc.vector.tensor_scalar(
                    d2[hs, rh * W:(rh + 1) * W], bvf[hs, :],
                    at[hs, rh:rh + 1], None, AluOpType.add,
                )
            nc.scalar.activation(dist[hs, :], d2[hs, :], AF.Sqrt)
            nc.scalar.activation(sig[hs, :], dist[hs, :], AF.Copy,
                                 bias=0.01, scale=0.99)
            nc.scalar.activation(sqv[hs, :], sig[hs, :], AF.Square)
            nc.vector.reciprocal_approx_fast(isg[:], sqv[:])
            nc.scalar.activation(u1f[hs, :], isg[hs, :], AF.Exp, scale=-0.5)
            nc.scalar.activation(u4f[hs, :], isg[hs, :], AF.Exp, scale=-2.0)
            nc.scalar.activation(u9f[hs, :], isg[hs, :], AF.Exp, scale=-4.5)
            for e in ALL_E:
                nc.scalar.activation(ub[e][hs, :], isg[hs, :], AF.Exp,
                                     scale=-0.5 * e)
            nc.vector.tensor_tensor(t1[hs, :], u1f[hs, :], u4f[hs, :],
                                    AluOpType.add)
            nc.vector.tensor_tensor(t2[hs, :], t1[hs, :], u9f[hs, :],
                                    AluOpType.add)
            nc.vector.tensor_scalar(
                sfield[hs, :], t2[hs, :], 2.0, 1.0,
                AluOpType.mult, AluOpType.add
            )
            nc.vector.reciprocal_approx_fast(rsf[:], sfield[:])
            nc.scalar.activation(rb[hs, :], rsf[hs, :], AF.Square)

            # ---------------- gaussian modulation (full width) --------------
            def gauss():
                P0, PN = 0, NP

                def xsv(col_off):
                    return _v(xs[:], P0 * XFS + PG * WP + PG + col_off,
                              [[XFS, PN], [SR * WP, C], [WP, 2], [1, W]])

                def rpv(a, col_off):
                    return _v(rp[a][:], P0 * RFS + PG + col_off,
                              [[RFS, PN], [2 * WP, C], [WP, 2], [1, W]])

                def pv(t):
                    return _v(t[:], P0 * PFS, [[PFS, PN], [1, PFS]])

                def pv4(t):
                    return _v(t[:], P0 * PFS,
                              [[PFS, PN], [2 * W, C], [W, 2], [1, W]])

                def uv(t):
                    return _v(t[:], P0 * CFS, [[CFS, PN], [0, C], [1, CFS]])

                hs = slice(P0, P0 + PN)

                for a in (1, 2, 3):
                    i0 = _v(xs[:], P0 * XFS + (PG - a) * WP,
                            [[XFS, PN], [SR * WP, C], [1, 2 * WP]])
                    i1 = _v(xs[:], P0 * XFS + (PG + a) * WP,
                            [[XFS, PN], [SR * WP, C], [1, 2 * WP]])
                    o = _v(rp[a][:], P0 * RFS, [[RFS, PN], [1, RFS]])
                    nc.vector.tensor_tensor(o, i0, i1, AluOpType.add)

                for (a, b), pt in ptiles.items():
                    if b == 0:
                        continue
                    if a == 0:
                        i0, i1 = xsv(-b), xsv(+b)
                    else:
                        i0, i1 = rpv(a, -b), rpv(a, +b)
                    eng = nc.gpsimd if (a, b) in ((0, 2), (0, 3)) else nc.vector
                    eng.tensor_tensor(pv4(pt), i0, i1, AluOpType.add)

                for e, (a, b) in EXPS_PAIR.items():
                    if a == 0:
                        nc.vector.tensor_tensor(
                            pv4(qtiles[e]), pv4(ptiles[(a, b)]), rpv(b, 0),
                            AluOpType.add,
                        )
                    else:
                        nc.vector.tensor_tensor(
                            pv(qtiles[e]), pv(ptiles[(a, b)]),
                            pv(ptiles[(b, a)]), AluOpType.add,
                        )

                sto = {1: qtiles[1], 2: ptiles[(1, 1)], 4: qtiles[4],
                       5: qtiles[5], 8: ptiles[(2, 2)], 9: qtiles[9]}
                prt = {1: prod, 2: acc, 4: acc2, 5: npr5, 8: npr8, 9: npr9}
                for e in ALL_E:
                    nc.vector.tensor_tensor(pv(prt[e]), uv(ub[e]), pv(sto[e]),
                                            AluOpType.mult)
                nc.vector.tensor_tensor(pv(qtiles[1]), pv(prt[1]), pv(prt[2]),
                                        AluOpType.add)
                nc.vector.tensor_tensor(pv(qtiles[4]), pv(prt[4]), pv(prt[5]),
                                        AluOpType.add)
                nc.vector.tensor_tensor(pv4(qtiles[5]), xsv(0), pv4(prt[8]),
                                        AluOpType.add)
                nc.vector.tensor_tensor(pv(qtiles[9]), pv(qtiles[1]),
                                        pv(qtiles[4]), AluOpType.add)
                nc.vector.tensor_tensor(pv(ptiles[(1, 1)]), pv(qtiles[5]),
                                        pv(prt[9]), AluOpType.add)
                nc.vector.tensor_tensor(pv(prod), pv(qtiles[9]),
                                        pv(ptiles[(1, 1)]), AluOpType.add)

                # m row pairs into mt2, (ci, rh, col) free order
                mdst = _v(mt2[:], P0 * MFS + PC,
                          [[MFS, PN], [2 * MW, C], [MW, 2], [1, W]])
                nc.vector.tensor_tensor(mdst, uv(rb), pv(prod), AluOpType.mult)

            # graduated strips: small first strips for a fast ramp after the
            # gaussian phase, 56-row strips for low steady-state overhead
            STRIPS = [(0, 28), (28, 28), (56, 56), (112, 56), (168, 56)]

            def emit_icf(r0, nrows):
                # one-hop im2col: icf[p=(dyp*C+ci)*NDXG+dxg] <- one contiguous
                # run of channel-major mf. 9 DMAs (per ci,dxg): SBUF dst hops
                # partitions only in dim 0 (stride 9 partitions over dyp);
                # issues split across sync and gpsimd queues.
                fs = nrows * MW
                runf = (nrows - 2) * MW + W
                icf = icf_pool.tile([K54, fs], BF16, name="icf")
                for ci in range(C):
                    for dxg in range(NDXG):
                        q = nc.sync if (ci * NDXG + dxg) % 2 == 0 else nc.gpsimd
                        q.dma_start(
                            out=_v(icf[:], (ci * NDXG + dxg) * fs,
                                   [[C * NDXG * fs, NDY], [1, runf + 1]]),
                            in_=_v(mf[:],
                                   (ci * MROWS + r0) * MW + 2 * dxg,
                                   [[MW, NDY], [1, runf + 1]]),
                        )
                return icf

            def conv_strip(icf, r0, nrows):
                fs = nrows * MW
                for h2 in range(nrows // 28):
                    stage = stg_pool.tile([2 * OC, FSST], BF16, name="stage")
                    for ch in range(7):
                        c = h2 * 7 + ch
                        ps = psum_pool.tile([2 * OC, NMM], F32, name="ps")
                        rhs0 = _v(icf[:], 4 * c * MW,
                                  [[fs, K54], [2 * MW, 2], [1, W]])
                        rhs1 = _v(icf[:], 4 * c * MW + 1,
                                  [[fs, K54], [2 * MW, 2], [1, W]])
                        nc.tensor.matmul(ps[:], wtA[:], rhs0,
                                         start=True, stop=False)
                        nc.tensor.matmul(ps[:], wtB[:], rhs1,
                                         start=False, stop=True)
                        st_slice = stage[:, ch * NMM:(ch + 1) * NMM]
                        if ch % 2 == 0:
                            nc.scalar.copy(st_slice, ps[:])
                        else:
                            nc.vector.tensor_scalar(
                                st_slice, ps[:], 1.0, None, AluOpType.mult
                            )

                    rp0 = r0 // 2 + h2 * 14
                    for h in range(2):
                        dst = _v(out[:],
                                 h * OC * (H // 2) * W + rp0 * W,
                                 [[(H // 2) * W, OC], [1, FSST]])
                        src = _v(stage[:], h * OC * FSST,
                                 [[FSST, OC], [1, FSST]])
                        nc.scalar.dma_start(out=dst, in_=src)

            gauss()

            # mt2 -> channel-major mf rows 2..229 (incl. zero bottom pad from
            # partitions 112,113); per-ci so each partition's (rh,col) block
            # is one contiguous 456-elem run
            for ci in range(C):
                nc.sync.dma_start(
                    out=_v(mf[:], (ci * MROWS + 2) * MW,
                           [[2 * MW, NP + 2], [1, 2 * MW]]),
                    in_=_v(mt2[:], ci * 2 * MW,
                           [[MFS, NP + 2], [1, 2 * MW]]),
                )

            for r0, nr in STRIPS:
                conv_strip(emit_icf(r0, nr), r0, nr)

    return nc


def _get_nc():
    global _CACHED
    if _CACHED is None:
        nc = _build_nc()
        nc.finalize()
        _CACHED = nc
    return _CACHED


def _host_prep(input_data, foa_xy, weight):
    b = input_data.shape[0]
    wcs = np.zeros((2 * K54, 2 * OC), dtype=np.float32)
    for ci in range(C):
        for dyp in range(NDY):
            for dxg in range(NDXG):
                k = (dyp * C + ci) * NDXG + dxg
                if dyp <= 4:
                    wcs[k, :OC] = weight[:, ci, dyp, 2 * dxg]
                    if dxg <= 1:
                        wcs[K54 + k, :OC] = weight[:, ci, dyp, 2 * dxg + 1]
                if dyp >= 1:
                    wcs[k, OC:] = weight[:, ci, dyp - 1, 2 * dxg]
                    if dxg <= 1:
                        wcs[K54 + k, OC:] = weight[:, ci, dyp - 1, 2 * dxg + 1]
    wcs = wcs.astype(ml_dtypes.bfloat16)
    idx = np.arange(H, dtype=np.float64)
    in_maps = []
    for i in range(b):
        xpad = np.zeros((C, WP, WP), dtype=ml_dtypes.bfloat16)
        xpad[:, PG:PG + H, PG:PG + W] = input_data[i].astype(ml_dtypes.bfloat16)
        fx, fy = float(foa_xy[i, 0]), float(foa_xy[i, 1])
        a_sq = (((idx - fx) / DNORM) ** 2).astype(np.float32)
        b_sq = (((idx - fy) / DNORM) ** 2).astype(np.float32)
        in_maps.append({"xp": xpad, "av": a_sq, "bv": b_sq, "wc": wcs})
    return in_maps


def kernel(input_data, foa_xy, weight):
    global LAST_RESULTS
    nc = _get_nc()
    in_maps = _host_prep(np.asarray(input_data), np.asarray(foa_xy),
                         np.asarray(weight))
    trace = bool(int(os.environ.get("BASSKERNEL_TRACE", "0")))
    res = run_bass_kernel_spmd(nc, in_maps, core_ids=list(range(8)), trace=trace)
    LAST_RESULTS = res
    outs = []
    for r in res.results:
        o2 = np.asarray(r["out"], dtype=np.float32)  # [2, OC, H/2, W]
        full = np.empty((OC, H, W), dtype=np.float32)
        full[:, 0::2, :] = o2[0]
        full[:, 1::2, :] = o2[1]
        outs.append(full)
    return np.stack(outs, axis=0)
